# revision 17
# baseline (speedup 1.0000x reference)
"""Multi-head attention (B=2, S=2048, D=1024, H=16) on 8 Trainium2 NeuronCores.

Sharding: core c -> (batch b = c // 4, head-group hg = c % 4, 4 heads each).
Each core computes its 4 heads' attention for its batch plus the partial
output projection (rows of w_o.T for its head dims). Host sums the 4 partial
outputs per batch and adds the bias constants.

All heavy layout work (transposes, weight slicing) is done host-side so the
device kernel is pure matmul / softmax dataflow:
  qhT/khT = W_h @ X^T          (transposed projections, [256, 2048])
  v       = X @ W_v^T          (natural layout, with ones column appended)
  scoresT = khT_h^T-slices: s^T[k, q] via PE (head pairs row-packed)
  exp     = ACT, scale=1/8 folded in, no max subtraction (scores ~ N(0,1))
  attnV   = [V | 1]^T @ exp  -> [65, 512] (row 64 = softmax sums)
  yT_part = W_o-slice^T-chunks @ aoT
Matmul dtype float32r (1 cyc/row on PE, ~1e-4 relative error).
"""
import sys

sys.path.insert(0, "/opt/trn_rl_repo")

import numpy as np

N_CORES = 8
B, S, D = 2, 2048, 1024
H, DH = 16, 64
HPC = H // 4  # head-groups
DLOC = D // 4  # 256 head dims per core
QT = 512  # q tile (moving dim)
NQT = S // QT  # 4
KT = 128  # k positions per scores tile
NKT = S // KT  # 16
KC = D // 128  # 8 contraction chunks for projections

_CACHE = {}


def _build():
    from concourse import bacc
    import concourse.mybir as mybir
    import concourse.tile as tile

    f32 = mybir.dt.float32
    f32r = mybir.dt.float32r
    AF = mybir.ActivationFunctionType

    nc = bacc.Bacc("TRN2", target_bir_lowering=False, debug=False,
                   num_devices=N_CORES)
    xq = nc.declare_dram_parameter("xq", [D, S], f32r, isOutput=False)
    xk = nc.declare_dram_parameter("xk", [D, S], f32r, isOutput=False)
    xv = nc.declare_dram_parameter("xv", [D, S], f32r, isOutput=False)
    wq = nc.declare_dram_parameter("wq", [D, DLOC], f32r, isOutput=False)
    wk = nc.declare_dram_parameter("wk", [D, DLOC], f32r, isOutput=False)
    wv = nc.declare_dram_parameter("wv", [D, DLOC], f32r, isOutput=False)
    wo = nc.declare_dram_parameter("wo", [DLOC, D], f32r, isOutput=False)
    ones64 = nc.declare_dram_parameter("ones64", [1, 64], f32r, isOutput=False)
    bq = nc.declare_dram_parameter("bq", [128, 2], f32, isOutput=False)
    bk = nc.declare_dram_parameter("bk", [128, 2], f32, isOutput=False)
    yT = nc.declare_dram_parameter("yT", [D, S], f32, isOutput=True)

    with tile.TileContext(nc) as tc:
        with (
            tc.tile_pool(name="keep", bufs=1) as keep,
            tc.tile_pool(name="big", bufs=1) as big,
            tc.tile_pool(name="small", bufs=2) as small,
            tc.tile_pool(name="ypool", bufs=3) as ypool,
            tc.tile_pool(name="psR", bufs=1, space="PSUM") as psR,
        ):
            # --- resident weights / constants ---
            wo_t = keep.tile([128, 2, D], f32r)
            nc.sync.dma_start(
                out=wo_t, in_=wo[:, :].rearrange("(c p) m -> p c m", p=128))
            bq_t = keep.tile([128, 2], f32)
            bk_t = keep.tile([128, 2], f32)
            nc.sync.dma_start(out=bq_t, in_=bq[:, :])
            nc.sync.dma_start(out=bk_t, in_=bk[:, :])

            ones1 = keep.tile([1, 64], f32r)
            nc.sync.dma_start(out=ones1, in_=ones64[:, :])
            # projections output (resident through attention)
            qh = big.tile([128, 2, S], f32r)   # [part, mt, q]
            kh = big.tile([128, 2, S], f32r)
            vsb = big.tile([128, NKT, 4, 65], f32r)  # [kpart, kt, head, d|1]
            aoT = big.tile([128, 2, S], f32r)  # attn out^T [dlocal, q]
            _o = ones64[:, :]
            import concourse.bass as bass_mod
            nc.sync.dma_start(
                out=vsb[:, :, :, 64:65],
                in_=bass_mod.AP(tensor=_o.tensor, offset=_o.offset,
                                ap=[[0, 128], [0, NKT * 4], [1, 1]]))

            # --- phase 1: projections ---
            with (
                tc.tile_pool(name="wpool", bufs=1) as wpool,
                tc.tile_pool(name="xpool", bufs=3) as xpool,
                tc.tile_pool(name="pp", bufs=2, space="PSUM") as pp,
            ):
                wq_t = wpool.tile([128, KC, DLOC], f32r, tag="wq")
                wk_t = wpool.tile([128, KC, DLOC], f32r, tag="wk")
                wv_t = wpool.tile([128, KC, DLOC], f32r, tag="wv")
                for w_t, w_d in ((wq_t, wq), (wk_t, wk), (wv_t, wv)):
                    nc.sync.dma_start(
                        out=w_t,
                        in_=w_d[:, :].rearrange("(c p) m -> p c m", p=128))

                for x_d, w_t, o_t, b_t in (
                    (xq, wq_t, qh, bq_t),
                    (xk, wk_t, kh, bk_t),
                ):
                    for nt in range(NQT):
                        xc = xpool.tile([128, KC, QT], f32r, tag="xc")
                        _xr = x_d[:, :].rearrange("(c p) s -> p c s", p=128)
                        nc.sync.dma_start(
                            out=xc[:, 0:KC // 2, :],
                            in_=_xr[:, 0:KC // 2, nt * QT:(nt + 1) * QT])
                        nc.sync.dma_start(
                            out=xc[:, KC // 2:KC, :],
                            in_=_xr[:, KC // 2:KC, nt * QT:(nt + 1) * QT])
                        for mt in range(2):
                            ps = pp.tile([128, QT], f32, tag="p")
                            for c in range(KC):
                                nc.tensor.matmul(
                                    ps,
                                    w_t[:, c, mt * 128:(mt + 1) * 128],
                                    xc[:, c, :],
                                    start=(c == 0), stop=(c == KC - 1))
                            nc.vector.tensor_scalar_add(
                                o_t[:, mt, nt * QT:(nt + 1) * QT],
                                ps, b_t[:, mt:mt + 1])

                # V projection, natural layout, per-head slices into vsb
                for nt in range(NQT):
                    xc = xpool.tile([128, KC, QT], f32r, tag="xc")
                    _xr = xv[:, :].rearrange("(c p) s -> p c s", p=128)
                    nc.sync.dma_start(
                        out=xc[:, 0:KC // 2, :],
                        in_=_xr[:, 0:KC // 2, nt * QT:(nt + 1) * QT])
                    nc.sync.dma_start(
                        out=xc[:, KC // 2:KC, :],
                        in_=_xr[:, KC // 2:KC, nt * QT:(nt + 1) * QT])
                    for stl in range(QT // 128):
                        st = nt * (QT // 128) + stl
                        psv = pp.tile([128, DLOC], f32, tag="pv")
                        for c in range(KC):
                            nc.tensor.matmul(
                                psv,
                                xc[:, c, stl * 128:(stl + 1) * 128],
                                wv_t[:, c, :],
                                start=(c == 0), stop=(c == KC - 1))
                        nc.vector.tensor_copy(
                            vsb[:, st, :, 0:64],
                            psv[:].rearrange("p (h d) -> p h d", h=4))

            # --- phase 2: attention ---
            with (
                tc.tile_pool(name="att", bufs=1) as att,
                tc.tile_pool(name="psS", bufs=2, space="PSUM") as psS,
                tc.tile_pool(name="psA", bufs=1, space="PSUM") as psA,
            ):
                def make_normalize(hp, qt, av0, av1):
                    def _norm():
                        for side, av in ((0, av0), (1, av1)):
                            rcp = small.tile([1, QT], f32r, tag="rcp")
                            with nc.allow_low_precision(
                                    reason="float32r has float32 bits"):
                                nc.vector.reciprocal(rcp, av[64:65, :])
                            rep = psR.tile([64, QT], f32, tag="rep")
                            nc.tensor.matmul(rep, ones1, rcp,
                                             start=True, stop=True)
                            bca = small.tile([64, QT], f32r, tag="bca")
                            nc.vector.tensor_copy(bca, rep)
                            if side == 0:
                                nc.vector.tensor_mul(
                                    aoT[0:64, hp, qt * QT:(qt + 1) * QT],
                                    av[0:64, :], bca)
                            else:
                                scr = small.tile([64, QT], f32r, tag="scr")
                                nc.vector.tensor_mul(scr, av[0:64, :], bca)
                                nc.sync.dma_start(
                                    out=aoT[64:128, hp,
                                            qt * QT:(qt + 1) * QT],
                                    in_=scr)
                    return _norm

                pending = None
                for hp in range(2):  # head pairs (2h, 2h+1), chunk = hp
                    for qt in range(NQT):
                        esb = att.tile([128, NKT, 2, QT], f32r, tag="esb")
                        av0 = psA.tile([128, QT], f32, tag="av0")
                        av1 = psA.tile([128, QT], f32, tag="av1")
                        for kt in range(NKT):
                            sc = psS.tile([128, 2 * QT], f32, tag="sc")
                            nc.tensor.matmul(
                                sc[:, 0:QT],
                                kh[0:64, hp, kt * 128:(kt + 1) * 128],
                                qh[0:64, hp, qt * QT:(qt + 1) * QT],
                                start=True, stop=True)
                            nc.tensor.matmul(
                                sc[:, QT:2 * QT],
                                kh[64:128, hp, kt * 128:(kt + 1) * 128],
                                qh[64:128, hp, qt * QT:(qt + 1) * QT],
                                start=True, stop=True)
                            nc.scalar.activation(
                                esb[:, kt, :, :], sc, AF.Exp, scale=0.125)
                            nc.tensor.matmul(
                                av0[0:65, :], vsb[:, kt, 2 * hp, :],
                                esb[:, kt, 0, :],
                                start=(kt == 0), stop=(kt == NKT - 1),
                                skip_group_check=True)
                            nc.tensor.matmul(
                                av1[0:65, :], vsb[:, kt, 2 * hp + 1, :],
                                esb[:, kt, 1, :],
                                start=(kt == 0), stop=(kt == NKT - 1),
                                skip_group_check=True)
                            if kt == 2 and pending is not None:
                                pending()
                                pending = None
                        pending = make_normalize(hp, qt, av0, av1)
                pending()

            # --- phase 3: output projection (partial) ---
            with tc.tile_pool(name="psY", bufs=2, space="PSUM") as psY:
                for mt in range(8):
                    for nt in range(NQT):
                        py = psY.tile([128, QT], f32, tag="py")
                        for c in range(2):
                            nc.tensor.matmul(
                                py,
                                wo_t[:, c, mt * 128:(mt + 1) * 128],
                                aoT[:, c, nt * QT:(nt + 1) * QT],
                                start=(c == 0), stop=(c == 1))
                        ysb = ypool.tile([128, QT], f32, tag="y")
                        nc.vector.tensor_copy(ysb, py)
                        nc.sync.dma_start(
                            out=yT[mt * 128:(mt + 1) * 128,
                                   nt * QT:(nt + 1) * QT],
                            in_=ysb)
    nc.compile()
    return nc


def _get_nc():
    if "nc" not in _CACHE:
        _CACHE["nc"] = _build()
    return _CACHE["nc"]


def kernel(q, k, v, w_q, b_q, w_k, b_k, w_v, b_v, w_o, b_o, _trace=False):
    from concourse.bass_utils import run_bass_kernel_spmd

    q = np.asarray(q, np.float32)
    k = np.asarray(k, np.float32)
    v = np.asarray(v, np.float32)
    w_q = np.asarray(w_q, np.float32)
    w_k = np.asarray(w_k, np.float32)
    w_v = np.asarray(w_v, np.float32)
    w_o = np.asarray(w_o, np.float32)
    b_q = np.asarray(b_q, np.float32)
    b_k = np.asarray(b_k, np.float32)
    b_v = np.asarray(b_v, np.float32)
    b_o = np.asarray(b_o, np.float32)

    nc = _get_nc()

    xqT = [np.ascontiguousarray(q[b].T) for b in range(B)]
    xkT = [np.ascontiguousarray(k[b].T) for b in range(B)]
    xvT = [np.ascontiguousarray(v[b].T) for b in range(B)]

    in_maps = []
    for c in range(N_CORES):
        b, hg = c // 4, c % 4
        lo, hi = hg * DLOC, (hg + 1) * DLOC
        in_maps.append({
            "xq": xqT[b],
            "xk": xkT[b],
            "xv": xvT[b],
            "wq": np.ascontiguousarray(w_q[lo:hi, :].T),
            "wk": np.ascontiguousarray(w_k[lo:hi, :].T),
            "wv": np.ascontiguousarray(w_v[lo:hi, :].T),
            "wo": np.ascontiguousarray(w_o[:, lo:hi].T),
            "ones64": np.ones((1, 64), np.float32),
            "bq": np.ascontiguousarray(b_q[lo:hi].reshape(2, 128).T),
            "bk": np.ascontiguousarray(b_k[lo:hi].reshape(2, 128).T),
        })

    res = run_bass_kernel_spmd(
        nc, in_maps, core_ids=list(range(N_CORES)), trace=_trace)
    if _trace:
        _CACHE["last_result"] = res

    # b_v contributes exactly (w_o @ b_v) per output element (softmax rows
    # sum to 1); b_o adds directly.
    const_row = (b_o + w_o @ b_v).astype(np.float32)  # [D]
    out = np.empty((B, S, D), np.float32)
    for b in range(B):
        acc = res.results[4 * b]["yT"].copy()
        for c in range(4 * b + 1, 4 * b + 4):
            acc += res.results[c]["yT"]
        out[b] = acc.T + const_row
    return out


# revision 18
# speedup vs baseline: 1.0221x; 1.0221x over previous
"""Multi-head attention (B=2, S=2048, D=1024, H=16) on 8 Trainium2 NeuronCores.

Sharding: core c -> (batch b = c // 4, head-group hg = c % 4, 4 heads each).
Each core computes its 4 heads' attention for its batch plus the partial
output projection (rows of w_o.T for its head dims). Host sums the 4 partial
outputs per batch and adds the bias constants.

All heavy layout work (transposes, weight slicing) is done host-side so the
device kernel is pure matmul / softmax dataflow:
  qhT/khT = W_h @ X^T          (transposed projections, [256, 2048])
  v       = X @ W_v^T          (natural layout, with ones column appended)
  scoresT = khT_h^T-slices: s^T[k, q] via PE (head pairs row-packed)
  exp     = ACT, scale=1/8 folded in, no max subtraction (scores ~ N(0,1))
  attnV   = [V | 1]^T @ exp  -> [65, 512] (row 64 = softmax sums)
  yT_part = W_o-slice^T-chunks @ aoT
Matmul dtype float32r (1 cyc/row on PE, ~1e-4 relative error).
"""
import sys

sys.path.insert(0, "/opt/trn_rl_repo")

import numpy as np
import ml_dtypes

N_CORES = 8
B, S, D = 2, 2048, 1024
H, DH = 16, 64
HPC = H // 4  # head-groups
DLOC = D // 4  # 256 head dims per core
QT = 512  # q tile (moving dim)
NQT = S // QT  # 4
KT = 128  # k positions per scores tile
NKT = S // KT  # 16
KC = D // 128  # 8 contraction chunks for projections

_CACHE = {}


def _build():
    from concourse import bacc
    import concourse.mybir as mybir
    import concourse.tile as tile

    f32 = mybir.dt.float32
    f32r = mybir.dt.float32r
    bf16 = mybir.dt.bfloat16
    AF = mybir.ActivationFunctionType

    nc = bacc.Bacc("TRN2", target_bir_lowering=False, debug=False,
                   num_devices=N_CORES)
    xq = nc.declare_dram_parameter("xq", [D, S], f32r, isOutput=False)
    xk = nc.declare_dram_parameter("xk", [D, S], f32r, isOutput=False)
    xv = nc.declare_dram_parameter("xv", [D, S], f32r, isOutput=False)
    wq = nc.declare_dram_parameter("wq", [D, DLOC], f32r, isOutput=False)
    wk = nc.declare_dram_parameter("wk", [D, DLOC], f32r, isOutput=False)
    wv = nc.declare_dram_parameter("wv", [D, DLOC], f32r, isOutput=False)
    wo = nc.declare_dram_parameter("wo", [DLOC, D], f32r, isOutput=False)
    ones64 = nc.declare_dram_parameter("ones64", [1, 64], f32r, isOutput=False)
    onesb = nc.declare_dram_parameter("onesb", [1, 64], bf16, isOutput=False)
    bq = nc.declare_dram_parameter("bq", [128, 2], f32, isOutput=False)
    bk = nc.declare_dram_parameter("bk", [128, 2], f32, isOutput=False)
    yT = nc.declare_dram_parameter("yT", [D, S], f32, isOutput=True)

    with tile.TileContext(nc) as tc:
        with (
            tc.tile_pool(name="keep", bufs=1) as keep,
            tc.tile_pool(name="big", bufs=1) as big,
            tc.tile_pool(name="small", bufs=2) as small,
            tc.tile_pool(name="ypool", bufs=3) as ypool,
            tc.tile_pool(name="psR", bufs=1, space="PSUM") as psR,
        ):
            # --- resident weights / constants ---
            wo_t = keep.tile([128, 2, D], f32r)
            nc.sync.dma_start(
                out=wo_t, in_=wo[:, :].rearrange("(c p) m -> p c m", p=128))
            bq_t = keep.tile([128, 2], f32)
            bk_t = keep.tile([128, 2], f32)
            nc.sync.dma_start(out=bq_t, in_=bq[:, :])
            nc.sync.dma_start(out=bk_t, in_=bk[:, :])

            ones1 = keep.tile([1, 64], f32r)
            nc.sync.dma_start(out=ones1, in_=ones64[:, :])
            # projections output (resident through attention)
            qh = big.tile([128, 2, S], bf16)   # [part, mt, q]
            kh = big.tile([128, 2, S], bf16)
            vsb = big.tile([128, NKT, 4, 65], bf16)  # [kpart, kt, head, d|1]
            aoT = big.tile([128, 2, S], f32r)  # attn out^T [dlocal, q]
            _o = onesb[:, :]
            import concourse.bass as bass_mod
            nc.sync.dma_start(
                out=vsb[:, :, :, 64:65],
                in_=bass_mod.AP(tensor=_o.tensor, offset=_o.offset,
                                ap=[[0, 128], [0, NKT * 4], [1, 1]]))

            # --- phase 1: projections ---
            with (
                tc.tile_pool(name="wpool", bufs=1) as wpool,
                tc.tile_pool(name="xpool", bufs=3) as xpool,
                tc.tile_pool(name="pp", bufs=2, space="PSUM") as pp,
            ):
                wq_t = wpool.tile([128, KC, DLOC], f32r, tag="wq")
                wk_t = wpool.tile([128, KC, DLOC], f32r, tag="wk")
                wv_t = wpool.tile([128, KC, DLOC], f32r, tag="wv")
                for w_t, w_d in ((wq_t, wq), (wk_t, wk), (wv_t, wv)):
                    nc.sync.dma_start(
                        out=w_t,
                        in_=w_d[:, :].rearrange("(c p) m -> p c m", p=128))

                for x_d, w_t, o_t, b_t in (
                    (xq, wq_t, qh, bq_t),
                    (xk, wk_t, kh, bk_t),
                ):
                    for nt in range(NQT):
                        xc = xpool.tile([128, KC, QT], f32r, tag="xc")
                        _xr = x_d[:, :].rearrange("(c p) s -> p c s", p=128)
                        nc.sync.dma_start(
                            out=xc[:, 0:KC // 2, :],
                            in_=_xr[:, 0:KC // 2, nt * QT:(nt + 1) * QT])
                        nc.sync.dma_start(
                            out=xc[:, KC // 2:KC, :],
                            in_=_xr[:, KC // 2:KC, nt * QT:(nt + 1) * QT])
                        for mt in range(2):
                            ps = pp.tile([128, QT], f32, tag="p")
                            for c in range(KC):
                                nc.tensor.matmul(
                                    ps,
                                    w_t[:, c, mt * 128:(mt + 1) * 128],
                                    xc[:, c, :],
                                    start=(c == 0), stop=(c == KC - 1))
                            nc.vector.tensor_scalar_add(
                                o_t[:, mt, nt * QT:(nt + 1) * QT],
                                ps, b_t[:, mt:mt + 1])

                # V projection, natural layout, per-head slices into vsb
                for nt in range(NQT):
                    xc = xpool.tile([128, KC, QT], f32r, tag="xc")
                    _xr = xv[:, :].rearrange("(c p) s -> p c s", p=128)
                    nc.sync.dma_start(
                        out=xc[:, 0:KC // 2, :],
                        in_=_xr[:, 0:KC // 2, nt * QT:(nt + 1) * QT])
                    nc.sync.dma_start(
                        out=xc[:, KC // 2:KC, :],
                        in_=_xr[:, KC // 2:KC, nt * QT:(nt + 1) * QT])
                    for stl in range(QT // 128):
                        st = nt * (QT // 128) + stl
                        psv = pp.tile([128, DLOC], f32, tag="pv")
                        for c in range(KC):
                            nc.tensor.matmul(
                                psv,
                                xc[:, c, stl * 128:(stl + 1) * 128],
                                wv_t[:, c, :],
                                start=(c == 0), stop=(c == KC - 1))
                        nc.vector.tensor_copy(
                            vsb[:, st, :, 0:64],
                            psv[:].rearrange("p (h d) -> p h d", h=4))

            # --- phase 2: attention ---
            with (
                tc.tile_pool(name="att", bufs=1) as att,
                tc.tile_pool(name="psS", bufs=2, space="PSUM") as psS,
                tc.tile_pool(name="psA", bufs=1, space="PSUM") as psA,
            ):
                for hp in range(2):  # head pairs (2h, 2h+1), chunk = hp
                    for qt in range(NQT):
                        esb = att.tile([128, NKT, 2, QT], bf16, tag="esb")
                        av0 = psA.tile([128, QT], f32, tag="av0")
                        av1 = psA.tile([128, QT], f32, tag="av1")
                        for kt in range(NKT):
                            sc = psS.tile([128, 2 * QT], f32, tag="sc")
                            nc.tensor.matmul(
                                sc[:, 0:QT],
                                kh[0:64, hp, kt * 128:(kt + 1) * 128],
                                qh[0:64, hp, qt * QT:(qt + 1) * QT],
                                start=True, stop=True)
                            nc.tensor.matmul(
                                sc[:, QT:2 * QT],
                                kh[64:128, hp, kt * 128:(kt + 1) * 128],
                                qh[64:128, hp, qt * QT:(qt + 1) * QT],
                                start=True, stop=True)
                            nc.scalar.activation(
                                esb[:, kt, :, :], sc, AF.Exp, scale=0.125)
                            nc.tensor.matmul(
                                av0[0:65, :], vsb[:, kt, 2 * hp, :],
                                esb[:, kt, 0, :],
                                start=(kt == 0), stop=(kt == NKT - 1),
                                skip_group_check=True)
                            nc.tensor.matmul(
                                av1[0:65, :], vsb[:, kt, 2 * hp + 1, :],
                                esb[:, kt, 1, :],
                                start=(kt == 0), stop=(kt == NKT - 1),
                                skip_group_check=True)
                        for side, av in ((0, av0), (1, av1)):
                            rcp = small.tile([1, QT], f32r, tag="rcp")
                            with nc.allow_low_precision(
                                    reason="float32r has float32 bits"):
                                nc.vector.reciprocal(rcp, av[64:65, :])
                            rep = psR.tile([64, QT], f32, tag="rep")
                            nc.tensor.matmul(rep, ones1, rcp,
                                             start=True, stop=True)
                            bca = small.tile([64, QT], f32r, tag="bca")
                            nc.vector.tensor_copy(bca, rep)
                            if side == 0:
                                nc.vector.tensor_mul(
                                    aoT[0:64, hp, qt * QT:(qt + 1) * QT],
                                    av[0:64, :], bca)
                            else:
                                scr = small.tile([64, QT], f32r, tag="scr")
                                nc.vector.tensor_mul(scr, av[0:64, :], bca)
                                nc.sync.dma_start(
                                    out=aoT[64:128, hp,
                                            qt * QT:(qt + 1) * QT],
                                    in_=scr)

            # --- phase 3: output projection (partial) ---
            with tc.tile_pool(name="psY", bufs=2, space="PSUM") as psY:
                for mt in range(8):
                    for nt in range(NQT):
                        py = psY.tile([128, QT], f32, tag="py")
                        for c in range(2):
                            nc.tensor.matmul(
                                py,
                                wo_t[:, c, mt * 128:(mt + 1) * 128],
                                aoT[:, c, nt * QT:(nt + 1) * QT],
                                start=(c == 0), stop=(c == 1))
                        ysb = ypool.tile([128, QT], f32, tag="y")
                        nc.vector.tensor_copy(ysb, py)
                        nc.sync.dma_start(
                            out=yT[mt * 128:(mt + 1) * 128,
                                   nt * QT:(nt + 1) * QT],
                            in_=ysb)
    nc.compile()
    return nc


def _get_nc():
    if "nc" not in _CACHE:
        _CACHE["nc"] = _build()
    return _CACHE["nc"]


def kernel(q, k, v, w_q, b_q, w_k, b_k, w_v, b_v, w_o, b_o, _trace=False):
    from concourse.bass_utils import run_bass_kernel_spmd

    q = np.asarray(q, np.float32)
    k = np.asarray(k, np.float32)
    v = np.asarray(v, np.float32)
    w_q = np.asarray(w_q, np.float32)
    w_k = np.asarray(w_k, np.float32)
    w_v = np.asarray(w_v, np.float32)
    w_o = np.asarray(w_o, np.float32)
    b_q = np.asarray(b_q, np.float32)
    b_k = np.asarray(b_k, np.float32)
    b_v = np.asarray(b_v, np.float32)
    b_o = np.asarray(b_o, np.float32)

    nc = _get_nc()

    xqT = [np.ascontiguousarray(q[b].T) for b in range(B)]
    xkT = [np.ascontiguousarray(k[b].T) for b in range(B)]
    xvT = [np.ascontiguousarray(v[b].T) for b in range(B)]

    in_maps = []
    for c in range(N_CORES):
        b, hg = c // 4, c % 4
        lo, hi = hg * DLOC, (hg + 1) * DLOC
        in_maps.append({
            "xq": xqT[b],
            "xk": xkT[b],
            "xv": xvT[b],
            "wq": np.ascontiguousarray(w_q[lo:hi, :].T),
            "wk": np.ascontiguousarray(w_k[lo:hi, :].T),
            "wv": np.ascontiguousarray(w_v[lo:hi, :].T),
            "wo": np.ascontiguousarray(w_o[:, lo:hi].T),
            "ones64": np.ones((1, 64), np.float32),
            "onesb": np.ones((1, 64), ml_dtypes.bfloat16),
            "bq": np.ascontiguousarray(b_q[lo:hi].reshape(2, 128).T),
            "bk": np.ascontiguousarray(b_k[lo:hi].reshape(2, 128).T),
        })

    res = run_bass_kernel_spmd(
        nc, in_maps, core_ids=list(range(N_CORES)), trace=_trace)
    if _trace:
        _CACHE["last_result"] = res

    # b_v contributes exactly (w_o @ b_v) per output element (softmax rows
    # sum to 1); b_o adds directly.
    const_row = (b_o + w_o @ b_v).astype(np.float32)  # [D]
    out = np.empty((B, S, D), np.float32)
    for b in range(B):
        acc = res.results[4 * b]["yT"].copy()
        for c in range(4 * b + 1, 4 * b + 4):
            acc += res.results[c]["yT"]
        out[b] = acc.T + const_row
    return out


# revision 20
# speedup vs baseline: 1.1788x; 1.1534x over previous
"""Multi-head attention (B=2, S=2048, D=1024, H=16) on 8 Trainium2 NeuronCores.

Sharding: core c -> (batch b = c // 4, head-group hg = c % 4, 4 heads each).
Each core computes its 4 heads' attention for its batch plus the partial
output projection (rows of w_o.T for its head dims). Host sums the 4 partial
outputs per batch and adds the bias constants.

All heavy layout work (transposes, weight slicing) is done host-side so the
device kernel is pure matmul / softmax dataflow:
  qhT/khT = W_h @ X^T          (transposed projections, [256, 2048])
  v       = X @ W_v^T          (natural layout, with ones column appended)
  scoresT = khT_h^T-slices: s^T[k, q] via PE (head pairs row-packed)
  exp     = ACT, scale=1/8 folded in, no max subtraction (scores ~ N(0,1))
  attnV   = [V | 1]^T @ exp  -> [65, 512] (row 64 = softmax sums)
  yT_part = W_o-slice^T-chunks @ aoT
Matmul dtype float32r (1 cyc/row on PE, ~1e-4 relative error).
"""
import sys

sys.path.insert(0, "/opt/trn_rl_repo")

import numpy as np

N_CORES = 8
B, S, D = 2, 2048, 1024
H, DH = 16, 64
HPC = H // 4  # head-groups
DLOC = D // 4  # 256 head dims per core
QT = 512  # q tile (moving dim)
NQT = S // QT  # 4
KT = 128  # k positions per scores tile
NKT = S // KT  # 16
KC = D // 128  # 8 contraction chunks for projections

_CACHE = {}


def _build():
    from concourse import bacc
    import concourse.mybir as mybir
    import concourse.tile as tile

    f32 = mybir.dt.float32
    f32r = mybir.dt.float32r
    AF = mybir.ActivationFunctionType

    nc = bacc.Bacc("TRN2", target_bir_lowering=False, debug=False,
                   num_devices=N_CORES)
    xq = nc.declare_dram_parameter("xq", [D, S], f32r, isOutput=False)
    xk = nc.declare_dram_parameter("xk", [D, S], f32r, isOutput=False)
    xv = nc.declare_dram_parameter("xv", [D, S], f32r, isOutput=False)
    wq = nc.declare_dram_parameter("wq", [D, DLOC], f32r, isOutput=False)
    wk = nc.declare_dram_parameter("wk", [D, DLOC], f32r, isOutput=False)
    wv = nc.declare_dram_parameter("wv", [D, DLOC], f32r, isOutput=False)
    wo = nc.declare_dram_parameter("wo", [DLOC, D], f32r, isOutput=False)
    ones64 = nc.declare_dram_parameter("ones64", [1, 64], f32r, isOutput=False)
    bq = nc.declare_dram_parameter("bq", [128, 2], f32, isOutput=False)
    bk = nc.declare_dram_parameter("bk", [128, 2], f32, isOutput=False)
    yT = nc.declare_dram_parameter("yT", [D, S], f32, isOutput=True)

    with tile.TileContext(nc) as tc:
        with (
            tc.tile_pool(name="keep", bufs=1) as keep,
            tc.tile_pool(name="big", bufs=1) as big,
            tc.tile_pool(name="small", bufs=2) as small,
            tc.tile_pool(name="ypool", bufs=3) as ypool,
        ):
            # --- resident weights / constants ---
            wo_t = keep.tile([128, 2, D], f32r)
            nc.sync.dma_start(
                out=wo_t, in_=wo[:, :].rearrange("(c p) m -> p c m", p=128))
            bq_t = keep.tile([128, 2], f32)
            bk_t = keep.tile([128, 2], f32)
            nc.sync.dma_start(out=bq_t, in_=bq[:, :])
            nc.sync.dma_start(out=bk_t, in_=bk[:, :])

            # projections output (resident through attention)
            qh = big.tile([128, 2, S], f32r)   # [part, mt, q]
            kh = big.tile([128, 2, S], f32r)
            vsb = big.tile([128, NKT, 4, 65], f32r)  # [kpart, kt, head, d|1]
            aoT = big.tile([128, 2, S], f32r)  # attn out^T [dlocal, q]
            _o = ones64[:, :]
            import concourse.bass as bass_mod
            nc.sync.dma_start(
                out=vsb[:, :, :, 64:65],
                in_=bass_mod.AP(tensor=_o.tensor, offset=_o.offset,
                                ap=[[0, 128], [0, NKT * 4], [1, 1]]))

            # --- phase 1: projections ---
            with (
                tc.tile_pool(name="wpool", bufs=1) as wpool,
                tc.tile_pool(name="xpool", bufs=3) as xpool,
                tc.tile_pool(name="pp", bufs=2, space="PSUM") as pp,
            ):
                wq_t = wpool.tile([128, KC, DLOC], f32r, tag="wq")
                wk_t = wpool.tile([128, KC, DLOC], f32r, tag="wk")
                wv_t = wpool.tile([128, KC, DLOC], f32r, tag="wv")
                for w_t, w_d in ((wq_t, wq), (wk_t, wk), (wv_t, wv)):
                    nc.sync.dma_start(
                        out=w_t,
                        in_=w_d[:, :].rearrange("(c p) m -> p c m", p=128))

                for x_d, w_t, o_t, b_t in (
                    (xq, wq_t, qh, bq_t),
                    (xk, wk_t, kh, bk_t),
                ):
                    for nt in range(NQT):
                        xc = xpool.tile([128, KC, QT], f32r, tag="xc")
                        _xr = x_d[:, :].rearrange("(c p) s -> p c s", p=128)
                        nc.sync.dma_start(
                            out=xc[:, 0:KC // 2, :],
                            in_=_xr[:, 0:KC // 2, nt * QT:(nt + 1) * QT])
                        nc.sync.dma_start(
                            out=xc[:, KC // 2:KC, :],
                            in_=_xr[:, KC // 2:KC, nt * QT:(nt + 1) * QT])
                        for mt in range(2):
                            ps = pp.tile([128, QT], f32, tag="p")
                            for c in range(KC):
                                nc.tensor.matmul(
                                    ps,
                                    w_t[:, c, mt * 128:(mt + 1) * 128],
                                    xc[:, c, :],
                                    start=(c == 0), stop=(c == KC - 1))
                            nc.vector.tensor_scalar_add(
                                o_t[:, mt, nt * QT:(nt + 1) * QT],
                                ps, b_t[:, mt:mt + 1])

                # V projection, natural layout, per-head slices into vsb
                for nt in range(NQT):
                    xc = xpool.tile([128, KC, QT], f32r, tag="xc")
                    _xr = xv[:, :].rearrange("(c p) s -> p c s", p=128)
                    nc.sync.dma_start(
                        out=xc[:, 0:KC // 2, :],
                        in_=_xr[:, 0:KC // 2, nt * QT:(nt + 1) * QT])
                    nc.sync.dma_start(
                        out=xc[:, KC // 2:KC, :],
                        in_=_xr[:, KC // 2:KC, nt * QT:(nt + 1) * QT])
                    for stl in range(QT // 128):
                        st = nt * (QT // 128) + stl
                        psv = pp.tile([128, DLOC], f32, tag="pv")
                        for c in range(KC):
                            nc.tensor.matmul(
                                psv,
                                xc[:, c, stl * 128:(stl + 1) * 128],
                                wv_t[:, c, :],
                                start=(c == 0), stop=(c == KC - 1))
                        nc.vector.tensor_copy(
                            vsb[:, st, :, 0:64],
                            psv[:].rearrange("p (h d) -> p h d", h=4))

            # --- phase 2: attention ---
            with (
                tc.tile_pool(name="att", bufs=1) as att,
                tc.tile_pool(name="psS", bufs=2, space="PSUM") as psS,
                tc.tile_pool(name="psA", bufs=2, space="PSUM") as psA,
            ):
                for hp in range(2):  # head pairs (2h, 2h+1), chunk = hp
                    for qt in range(NQT):
                        esb = att.tile([128, NKT, 2, QT], f32r, tag="esb")
                        av0 = psA.tile([128, QT], f32, tag="av0")
                        av1 = psA.tile([128, QT], f32, tag="av1")
                        for kt in range(NKT):
                            sc = psS.tile([128, 2 * QT], f32, tag="sc")
                            nc.tensor.matmul(
                                sc[:, 0:QT],
                                kh[0:64, hp, kt * 128:(kt + 1) * 128],
                                qh[0:64, hp, qt * QT:(qt + 1) * QT],
                                start=True, stop=True)
                            nc.tensor.matmul(
                                sc[:, QT:2 * QT],
                                kh[64:128, hp, kt * 128:(kt + 1) * 128],
                                qh[64:128, hp, qt * QT:(qt + 1) * QT],
                                start=True, stop=True)
                            nc.scalar.activation(
                                esb[:, kt, :, :], sc, AF.Exp, scale=0.125)
                            nc.tensor.matmul(
                                av0[0:65, :], vsb[:, kt, 2 * hp, :],
                                esb[:, kt, 0, :],
                                start=(kt == 0), stop=(kt == NKT - 1),
                                skip_group_check=True)
                            nc.tensor.matmul(
                                av1[0:65, :], vsb[:, kt, 2 * hp + 1, :],
                                esb[:, kt, 1, :],
                                start=(kt == 0), stop=(kt == NKT - 1),
                                skip_group_check=True)
                        for side, av in ((0, av0), (1, av1)):
                            rcp = small.tile([1, QT], f32, tag="rcp")
                            nc.vector.reciprocal(rcp, av[64:65, :])
                            bca = small.tile([64, QT], f32, tag="bca")
                            _rc = rcp[:]
                            nc.sync.dma_start(
                                out=bca,
                                in_=bass_mod.AP(
                                    tensor=_rc.tensor, offset=_rc.offset,
                                    ap=[[1, 1], [0, 64], [1, QT]]))
                            if side == 0:
                                nc.vector.tensor_mul(
                                    aoT[0:64, hp, qt * QT:(qt + 1) * QT],
                                    av[0:64, :], bca)
                            else:
                                scr = small.tile([64, QT], f32r, tag="scr")
                                nc.vector.tensor_mul(scr, av[0:64, :], bca)
                                nc.sync.dma_start(
                                    out=aoT[64:128, hp,
                                            qt * QT:(qt + 1) * QT],
                                    in_=scr)

            # --- phase 3: output projection (partial) ---
            with tc.tile_pool(name="psY", bufs=2, space="PSUM") as psY:
                for mt in range(8):
                    for nt in range(NQT):
                        py = psY.tile([128, QT], f32, tag="py")
                        for c in range(2):
                            nc.tensor.matmul(
                                py,
                                wo_t[:, c, mt * 128:(mt + 1) * 128],
                                aoT[:, c, nt * QT:(nt + 1) * QT],
                                start=(c == 0), stop=(c == 1))
                        ysb = ypool.tile([128, QT], f32, tag="y")
                        nc.vector.tensor_copy(ysb, py)
                        nc.sync.dma_start(
                            out=yT[mt * 128:(mt + 1) * 128,
                                   nt * QT:(nt + 1) * QT],
                            in_=ysb)
    nc.compile()
    return nc


def _get_nc():
    if "nc" not in _CACHE:
        _CACHE["nc"] = _build()
    return _CACHE["nc"]


def kernel(q, k, v, w_q, b_q, w_k, b_k, w_v, b_v, w_o, b_o, _trace=False):
    from concourse.bass_utils import run_bass_kernel_spmd

    q = np.asarray(q, np.float32)
    k = np.asarray(k, np.float32)
    v = np.asarray(v, np.float32)
    w_q = np.asarray(w_q, np.float32)
    w_k = np.asarray(w_k, np.float32)
    w_v = np.asarray(w_v, np.float32)
    w_o = np.asarray(w_o, np.float32)
    b_q = np.asarray(b_q, np.float32)
    b_k = np.asarray(b_k, np.float32)
    b_v = np.asarray(b_v, np.float32)
    b_o = np.asarray(b_o, np.float32)

    nc = _get_nc()

    xqT = [np.ascontiguousarray(q[b].T) for b in range(B)]
    xkT = [np.ascontiguousarray(k[b].T) for b in range(B)]
    xvT = [np.ascontiguousarray(v[b].T) for b in range(B)]

    in_maps = []
    for c in range(N_CORES):
        b, hg = c // 4, c % 4
        lo, hi = hg * DLOC, (hg + 1) * DLOC
        in_maps.append({
            "xq": xqT[b],
            "xk": xkT[b],
            "xv": xvT[b],
            "wq": np.ascontiguousarray(w_q[lo:hi, :].T),
            "wk": np.ascontiguousarray(w_k[lo:hi, :].T),
            "wv": np.ascontiguousarray(w_v[lo:hi, :].T),
            "wo": np.ascontiguousarray(w_o[:, lo:hi].T),
            "ones64": np.ones((1, 64), np.float32),
            "bq": np.ascontiguousarray(b_q[lo:hi].reshape(2, 128).T),
            "bk": np.ascontiguousarray(b_k[lo:hi].reshape(2, 128).T),
        })

    res = run_bass_kernel_spmd(
        nc, in_maps, core_ids=list(range(N_CORES)), trace=_trace)
    if _trace:
        _CACHE["last_result"] = res

    # b_v contributes exactly (w_o @ b_v) per output element (softmax rows
    # sum to 1); b_o adds directly.
    const_row = (b_o + w_o @ b_v).astype(np.float32)  # [D]
    out = np.empty((B, S, D), np.float32)
    for b in range(B):
        acc = res.results[4 * b]["yT"].copy()
        for c in range(4 * b + 1, 4 * b + 4):
            acc += res.results[c]["yT"]
        out[b] = acc.T + const_row
    return out
